# revision 1
# baseline (speedup 1.0000x reference)
"""Trainium2 Bass kernel for upsample_conv_2d (conv_transpose stride-2 3x3 +
4x4 FIR + bias), data-parallel over batch on 8 NeuronCores.

Math: conv_transpose(x, w, stride 2) followed by the 4x4 FIR is a single
linear convolution with a composed 6x6 kernel on the stride-2-upsampled
grid. Phase-decomposing by output parity (h%2, w%2) turns it into FOUR
independent 3x3 same-padding convolutions on the original 64x64 grid, all
sharing the same x windows:

    out[co, 2r+pa, 2s+pb] = bias[co]
        + sum_{ci,e,f} x[ci, r+e, s+f] * K[pa,pb][co, ci, e, f]

with K[pa,pb][e,f] = G[2e-pa, 2f-pb] and
G[d1,d2] = sum_{p-u=d1, q-v=d2} w[p,q] * fir[u,v].

Each phase conv is pure channel-contraction matmul work on the PE:
lhsT = K slice [ci(128), co(128)], rhs = shifted x window [ci(128), 8h x 64w],
accumulated over 9 taps x 2 ci-blocks into PSUM, bias added on the ACT
engine during the PSUM->SBUF copy.
"""

import json

import numpy as np

import concourse.bass as bass
import concourse.mybir as mybir
import concourse.tile as tile
from concourse.bass_utils import run_bass_kernel_spmd

# ---------------------------------------------------------------------------
# BIR post-pass: this walrus build rejects instructions carrying more than one
# sem wait (e.g. Tile's kernel-tail Drain gets 3). Hoist extras into
# standalone EventSemaphore instructions right before the owner.
# ---------------------------------------------------------------------------
_MAX_WAITS = 1


def _split_waits(j: dict) -> dict:
    for fn in j.get("functions", []):
        for blk in fn.get("blocks", []):
            insts = blk.get("instructions")
            if not insts:
                continue
            out = []
            for inst in insts:
                si = inst.get("sync_info") or {}
                waits = si.get("on_wait") or []
                if len(waits) > _MAX_WAITS:
                    for k, w in enumerate(waits[_MAX_WAITS:]):
                        out.append(
                            {
                                "debug": inst.get("debug", 0),
                                "engine": inst["engine"],
                                "ins": [],
                                "name": f"{inst['name']}-wsplit{k}",
                                "opcode": "EventSemaphore",
                                "outs": [],
                                "sync_info": {"on_update": [], "on_wait": [w]},
                            }
                        )
                    si["on_wait"] = waits[:_MAX_WAITS]
                out.append(inst)
            blk["instructions"] = out
    return j


_orig_to_json_bytes = bass.Bass.to_json_bytes


def _patched_to_json_bytes(self):
    return json.dumps(_split_waits(json.loads(_orig_to_json_bytes(self)))).encode()


bass.Bass.to_json_bytes = _patched_to_json_bytes

# ---------------------------------------------------------------------------
# Problem constants (hardcoded; kernel.py must be self-contained)
# ---------------------------------------------------------------------------
N, C, H, W = 8, 256, 64, 64
OH, OW = 2 * H, 2 * W
N_CORES = 8
F32 = mybir.dt.float32
F32R = mybir.dt.float32r

# tap order shared by host weight layout and device loop
_TAPS = [(e, f, cib) for e in (-1, 0, 1) for f in (-1, 0, 1) for cib in (0, 1)]


def _phase_weight_matrix(w: np.ndarray) -> np.ndarray:
    """[256,256,3,3] conv_transpose weight -> [128, 144*128] lhsT matrix.

    Column block index = ((cib*4 + ph)*9 + tap)*2 + cob, each 128 co wide;
    row = ci within ci-block. ph = pa*2+pb, tap = (e+1)*3+(f+1).
    """
    k1 = np.array([1.0, 3.0, 3.0, 1.0], dtype=np.float64)
    fir = np.outer(k1, k1)
    fir = fir / fir.sum() * 4.0  # gain = factor^2
    wd = w.astype(np.float64)
    # G[d1+3, d2+3] = sum_{p-u=d1, q-v=d2} w[p,q] fir[u,v]
    G = np.zeros((C, C, 6, 6), dtype=np.float64)
    for p in range(3):
        for q in range(3):
            for u in range(4):
                for v in range(4):
                    G[:, :, p - u + 3, q - v + 3] += wd[:, :, p, q] * fir[u, v]
    Wmat = np.zeros((128, 2 * 4 * 9 * 2, 128), dtype=np.float32)
    for cib in range(2):
        for pa in range(2):
            for pb in range(2):
                ph = pa * 2 + pb
                for e in (-1, 0, 1):
                    for f in (-1, 0, 1):
                        tap = (e + 1) * 3 + (f + 1)
                        # K[o, c] = G[o, c, 2e-pa+3, 2f-pb+3]
                        Kof = G[:, :, 2 * e - pa + 3, 2 * f - pb + 3]
                        for cob in range(2):
                            cidx = ((cib * 4 + ph) * 9 + tap) * 2 + cob
                            blk = Kof[
                                cob * 128 : (cob + 1) * 128,
                                cib * 128 : (cib + 1) * 128,
                            ]  # [co, ci]
                            Wmat[:, cidx, :] = blk.T.astype(np.float32)
    return Wmat.reshape(128, -1)


def _widx(cib: int, ph: int, tap: int, cob: int) -> int:
    return ((cib * 4 + ph) * 9 + tap) * 2 + cob


def build_nc(reps: int = 1) -> bass.Bass:
    nc = bass.Bass("TRN2", target_bir_lowering=False, debug=False)
    x_d = nc.dram_tensor("x", [C, H + 2, W + 2], F32R, kind="ExternalInput").ap()
    w_d = nc.dram_tensor("w", [128, 144 * 128], F32R, kind="ExternalInput").ap()
    b_d = nc.dram_tensor("bias", [2, 128], F32, kind="ExternalInput").ap()
    out_d = nc.dram_tensor("out", [C, OH, OW], F32, kind="ExternalOutput").ap()

    xb = x_d.rearrange("(b p) h w -> b p h w", p=128)

    with tile.TileContext(nc) as tc:
        with (
            tc.tile_pool(name="weights", bufs=1) as wpool,
            tc.tile_pool(name="xin", bufs=1) as xpool,
            tc.tile_pool(name="psum", bufs=8, space="PSUM") as ppool,
            tc.tile_pool(name="outs", bufs=3) as opool,
        ):
            wt = wpool.tile([128, 144, 128], F32R)
            nc.sync.dma_start(wt[:], w_d.rearrange("p (a b) -> p a b", b=128))
            bt = wpool.tile([128, 2], F32)
            nc.sync.dma_start(bt[:], b_d.rearrange("b p -> p b"))

            # x arrives zero-padded to 66x66 from the host
            xpad = [xpool.tile([128, H + 2, W + 2], F32R, tag=f"xp{i}", name=f"xp{i}") for i in range(2)]
            for cib in range(2):
                nc.sync.dma_start(xpad[cib][:], xb[cib])

            for _rep in range(reps):
                for half in range(2):
                    for cob in range(2):
                        out_tiles = [
                            opool.tile([128, 8, 2, 64, 2], F32, tag="ot", name="ot")
                            for _ in range(4)
                        ]
                        for pa in range(2):
                            for pb in range(2):
                                ph = pa * 2 + pb
                                psums = [
                                    ppool.tile([128, 8, 64], F32, tag="ps", name="ps")
                                    for _ in range(4)
                                ]
                                for it, (e, f, cib) in enumerate(_TAPS):
                                    tap = (e + 1) * 3 + (f + 1)
                                    lhsT = wt[:, _widx(cib, ph, tap, cob), :]
                                    for k in range(4):
                                        hb = half * 4 + k
                                        r0 = hb * 8 + 1 + e
                                        rhs = xpad[cib][
                                            :, r0 : r0 + 8, 1 + f : 65 + f
                                        ]
                                        nc.tensor.matmul(
                                            psums[k][:],
                                            lhsT,
                                            rhs,
                                            start=(it == 0),
                                            stop=(it == len(_TAPS) - 1),
                                        )
                                for k in range(4):
                                    nc.scalar.activation(
                                        out_tiles[k][:, :, pa, :, pb],
                                        psums[k][:],
                                        mybir.ActivationFunctionType.Identity,
                                        bias=bt[:, cob : cob + 1],
                                        scale=1.0,
                                    )
                        for k in range(4):
                            hb = half * 4 + k
                            dst = out_d[
                                cob * 128 : (cob + 1) * 128,
                                hb * 16 : hb * 16 + 16,
                                :,
                            ].rearrange("c (a b) (w v) -> c a b w v", b=2, v=2)
                            nc.sync.dma_start(dst, out_tiles[k][:])
    return nc


_CACHED_NC = {}


def _get_nc(reps: int = 1) -> bass.Bass:
    if reps not in _CACHED_NC:
        _CACHED_NC[reps] = build_nc(reps)
    return _CACHED_NC[reps]


def _run(x, weight, bias, reps: int = 1):
    Wmat = _phase_weight_matrix(np.asarray(weight, dtype=np.float32))
    b2 = np.ascontiguousarray(
        np.asarray(bias, dtype=np.float32).reshape(2, 128)
    )
    xs = np.pad(
        np.asarray(x, dtype=np.float32), ((0, 0), (0, 0), (1, 1), (1, 1))
    )
    nc = _get_nc(reps)
    in_maps = [
        {"x": xs[i], "w": Wmat, "bias": b2} for i in range(N_CORES)
    ]
    res = run_bass_kernel_spmd(nc, in_maps, list(range(N_CORES)))
    return np.stack([res.results[i]["out"] for i in range(N_CORES)])


def kernel(x, weight, bias):
    return _run(x, weight, bias, reps=1)



# revision 2
# speedup vs baseline: 3.4305x; 3.4305x over previous
"""Trainium2 Bass kernel v2 for upsample_conv_2d — fp16 datapath, big DMAs.

Same phase-decomposed math as the baseline (conv_transpose stride-2 3x3 +
4x4 FIR composed into four 3x3 convs on the 64x64 grid), but:
  - all SBUF tensors fp16 (PSUM accumulation stays fp32): matmul runs at
    full PE rate, LDWEIGHTS gets fast-weight-load, DMA bytes halve
  - output staged per (half, cob) as one [128, 64, 128] tile and shipped
    in 4 DMAs of 2MB with 16KB-contiguous-per-partition descriptors
    (the baseline's 16 x 1MB strided DMAs were the 61ms bottleneck)
  - DRAM output is fp16; the host upcasts to fp32 (rel-err budget 2e-2,
    fp16 rounding adds ~1e-3)
"""

import json

import numpy as np

import concourse.bass as bass
import concourse.mybir as mybir
import concourse.tile as tile
from concourse.bass_utils import run_bass_kernel_spmd

# ---------------------------------------------------------------------------
# BIR post-pass: this walrus build rejects instructions carrying more than one
# sem wait (e.g. Tile's kernel-tail Drain gets 3). Hoist extras into
# standalone EventSemaphore instructions right before the owner.
# ---------------------------------------------------------------------------
_MAX_WAITS = 1


def _split_waits(j: dict) -> dict:
    for fn in j.get("functions", []):
        for blk in fn.get("blocks", []):
            insts = blk.get("instructions")
            if not insts:
                continue
            out = []
            for inst in insts:
                si = inst.get("sync_info") or {}
                waits = si.get("on_wait") or []
                if len(waits) > _MAX_WAITS:
                    for k, w in enumerate(waits[_MAX_WAITS:]):
                        out.append(
                            {
                                "debug": inst.get("debug", 0),
                                "engine": inst["engine"],
                                "ins": [],
                                "name": f"{inst['name']}-wsplit{k}",
                                "opcode": "EventSemaphore",
                                "outs": [],
                                "sync_info": {"on_update": [], "on_wait": [w]},
                            }
                        )
                    si["on_wait"] = waits[:_MAX_WAITS]
                out.append(inst)
            blk["instructions"] = out
    return j


_orig_to_json_bytes = bass.Bass.to_json_bytes


def _patched_to_json_bytes(self):
    return json.dumps(_split_waits(json.loads(_orig_to_json_bytes(self)))).encode()


bass.Bass.to_json_bytes = _patched_to_json_bytes

# ---------------------------------------------------------------------------
# Problem constants (hardcoded; kernel.py must be self-contained)
# ---------------------------------------------------------------------------
N, C, H, W = 8, 256, 64, 64
OH, OW = 2 * H, 2 * W
N_CORES = 8
F32 = mybir.dt.float32
F16 = mybir.dt.float16

# tap order shared by host weight layout and device loop
_TAPS = [(e, f, cib) for e in (-1, 0, 1) for f in (-1, 0, 1) for cib in (0, 1)]


def _phase_weight_matrix(w: np.ndarray) -> np.ndarray:
    """[256,256,3,3] conv_transpose weight -> [128, 144*128] fp16 lhsT matrix.

    Column block index = ((cib*4 + ph)*9 + tap)*2 + cob, each 128 co wide;
    row = ci within ci-block. ph = pa*2+pb, tap = (e+1)*3+(f+1).
    """
    k1 = np.array([1.0, 3.0, 3.0, 1.0], dtype=np.float64)
    fir = np.outer(k1, k1)
    fir = fir / fir.sum() * 4.0  # gain = factor^2
    wd = w.astype(np.float64)
    # G[d1+3, d2+3] = sum_{p-u=d1, q-v=d2} w[p,q] fir[u,v]
    G = np.zeros((C, C, 6, 6), dtype=np.float64)
    for p in range(3):
        for q in range(3):
            for u in range(4):
                for v in range(4):
                    G[:, :, p - u + 3, q - v + 3] += wd[:, :, p, q] * fir[u, v]
    Wmat = np.zeros((128, 2 * 4 * 9 * 2, 128), dtype=np.float16)
    for cib in range(2):
        for pa in range(2):
            for pb in range(2):
                ph = pa * 2 + pb
                for e in (-1, 0, 1):
                    for f in (-1, 0, 1):
                        tap = (e + 1) * 3 + (f + 1)
                        # K[o, c] = G[o, c, 2e-pa+3, 2f-pb+3]
                        Kof = G[:, :, 2 * e - pa + 3, 2 * f - pb + 3]
                        for cob in range(2):
                            cidx = ((cib * 4 + ph) * 9 + tap) * 2 + cob
                            blk = Kof[
                                cob * 128 : (cob + 1) * 128,
                                cib * 128 : (cib + 1) * 128,
                            ]  # [co, ci]
                            Wmat[:, cidx, :] = blk.T.astype(np.float16)
    return Wmat.reshape(128, -1)


def _widx(cib: int, ph: int, tap: int, cob: int) -> int:
    return ((cib * 4 + ph) * 9 + tap) * 2 + cob


def build_nc(reps: int = 1, loop: bool = False) -> bass.Bass:
    nc = bass.Bass("TRN2", target_bir_lowering=False, debug=False)
    x_d = nc.dram_tensor("x", [C, H + 2, W + 2], F16, kind="ExternalInput").ap()
    w_d = nc.dram_tensor("w", [128, 144 * 128], F16, kind="ExternalInput").ap()
    b_d = nc.dram_tensor("bias", [2, 128], F32, kind="ExternalInput").ap()
    out_d = nc.dram_tensor("out", [C, OH, OW], F16, kind="ExternalOutput").ap()

    xb = x_d.rearrange("(b p) h w -> b p h w", p=128)

    with tile.TileContext(nc) as tc:
        with (
            tc.tile_pool(name="weights", bufs=1) as wpool,
            tc.tile_pool(name="xin", bufs=1) as xpool,
            tc.tile_pool(name="psum", bufs=8, space="PSUM") as ppool,
            tc.tile_pool(name="outs", bufs=4) as opool,
        ):
            wt = wpool.tile([128, 144, 128], F16)
            nc.sync.dma_start(wt[:], w_d.rearrange("p (a b) -> p a b", b=128))
            bt = wpool.tile([128, 2], F32)
            nc.sync.dma_start(bt[:], b_d.rearrange("b p -> p b"))

            # x arrives zero-padded to 66x66 from the host
            xpad = [
                xpool.tile([128, H + 2, W + 2], F16, tag=f"xp{i}", name=f"xp{i}")
                for i in range(2)
            ]
            for cib in range(2):
                nc.sync.dma_start(xpad[cib][:], xb[cib])

            def body():
                for half in range(2):
                    for cob in range(2):
                        T = opool.tile([128, 64, 128], F16, tag="st", name="st")
                        Tv = T[:].rearrange(
                            "p (k r a) (s b) -> p k r a s b", k=4, a=2, b=2
                        )
                        for pa in range(2):
                            for pb in range(2):
                                ph = pa * 2 + pb
                                psums = [
                                    ppool.tile([128, 8, 64], F32, tag="ps", name="ps")
                                    for _ in range(4)
                                ]
                                for it, (e, f, cib) in enumerate(_TAPS):
                                    tap = (e + 1) * 3 + (f + 1)
                                    lhsT = wt[:, _widx(cib, ph, tap, cob), :]
                                    for k in range(4):
                                        hb = half * 4 + k
                                        r0 = hb * 8 + 1 + e
                                        rhs = xpad[cib][
                                            :, r0 : r0 + 8, 1 + f : 65 + f
                                        ]
                                        nc.tensor.matmul(
                                            psums[k][:],
                                            lhsT,
                                            rhs,
                                            start=(it == 0),
                                            stop=(it == len(_TAPS) - 1),
                                        )
                                for k in range(4):
                                    nc.scalar.activation(
                                        Tv[:, k, :, pa, :, pb],
                                        psums[k][:],
                                        mybir.ActivationFunctionType.Identity,
                                        bias=bt[:, cob : cob + 1],
                                        scale=1.0,
                                    )
                        dst = out_d[
                            cob * 128 : (cob + 1) * 128,
                            half * 64 : half * 64 + 64,
                            :,
                        ].rearrange("c h w -> c (h w)")
                        nc.sync.dma_start(dst, T[:].rearrange("p h w -> p (h w)"))

            if loop:
                with tc.For_i(0, reps):
                    body()
            else:
                for _rep in range(reps):
                    body()
    return nc


_CACHED_NC = {}


def _get_nc(reps: int = 1, loop: bool = False) -> bass.Bass:
    key = (reps, loop)
    if key not in _CACHED_NC:
        _CACHED_NC[key] = build_nc(reps, loop)
    return _CACHED_NC[key]


def _run(x, weight, bias, reps: int = 1, loop: bool = False):
    Wmat = _phase_weight_matrix(np.asarray(weight, dtype=np.float32))
    b2 = np.ascontiguousarray(np.asarray(bias, dtype=np.float32).reshape(2, 128))
    xs = np.pad(
        np.asarray(x, dtype=np.float32), ((0, 0), (0, 0), (1, 1), (1, 1))
    ).astype(np.float16)
    nc = _get_nc(reps, loop)
    in_maps = [{"x": xs[i], "w": Wmat, "bias": b2} for i in range(N_CORES)]
    res = run_bass_kernel_spmd(nc, in_maps, list(range(N_CORES)))
    return np.stack(
        [res.results[i]["out"].astype(np.float32) for i in range(N_CORES)]
    )


def kernel(x, weight, bias):
    return _run(x, weight, bias, reps=1)
